# revision 6
# baseline (speedup 1.0000x reference)
"""GCN (3-layer) message-passing kernel for 8 Trainium2 NeuronCores.

Strategy (node sharding, receiver-major):
  - Nodes are sharded contiguously across 8 cores (6250 each). Each core
    aggregates messages for its own nodes only ("receivers").
  - Math identity used: with norm_w = ew * a[src] * b[tgt]
    (a = rsqrt(in_deg), b = rsqrt(out_deg)),
      out[n] = relu( a[n] * (sum_e ew_e * (b*h)[tgt_e]) @ W + a[n]*t[n]*bias ),
    where t[n] = sum_e ew_e*b[tgt_e].  We keep a per-node feature table whose
    rows are [b*h (96) | b | 0-pad] so the aggregated column 96 is exactly
    t[n], and the bias is folded as row 96 of a packed [97 x out] weight.
  - Per layer: each core rebuilds its table shard, an AllGather replicates the
    table, then per-core gathers (hardware dma_gather, 512B rows, int16
    indices over two table halves) feed PE matmuls against host-built
    one-hot "segment mask" tiles ([128 edges x 32 receiver slots], values =
    raw edge weight) accumulating segment sums in PSUM.
  - Receivers are bin-packed (host, index-only) into groups of <=31 nodes
    with <=256 edges per table half, so the SPMD program is fully static.
"""
import sys
sys.path.insert(0, "/opt/trn_rl_repo")
import numpy as np

# ---------------- problem constants (hardcoded per spec) ----------------
N = 50000
E = 800000
DIN = 96
DHID = 96
DOUT = 64
NC = 8
NPC = N // NC            # 6250 nodes per core

W_G = 32                 # receiver slots per group (mask width)
NODE_CAP = 31            # real nodes per group (slot 31 always dummy/zero row)
CAP = 256                # edge capacity per (group, half)
G_MAX = 224              # groups per core (multiple of 32)
RPC = G_MAX * W_G        # 7168 table rows per core
ROWS = NC * RPC          # 57344 total table rows
HALF = ROWS // 2         # 28672 (int16-addressable table halves)
S_PASS = G_MAX * CAP     # 57344 gather slots per pass (per half)
CALL = 8192              # gather slots per dma_gather call
NCALL = S_PASS // CALL   # 7
CSR_W = 48               # max node degree supported in degree CSRs
TILES = RPC // 128       # 56 row-tiles per core (== G_MAX // 4 clusters)
NCHUNK = CALL // 128     # 64 chunks per call
DUMMY_IDX = 31           # local idx of a guaranteed all-zero table row (both halves)

_CACHE = {}


# ======================= host-side graph packing =======================

def _pack(edge_src, edge_tgt, edge_weights):
    """Index-only preprocessing: group packing, gather/mask/CSR layouts."""
    src = np.asarray(edge_src).astype(np.int64)
    tgt = np.asarray(edge_tgt).astype(np.int64)
    ew = np.asarray(edge_weights).astype(np.float32)

    owner_src = src // NPC
    half_tgt = (tgt // NPC >= NC // 2).astype(np.int64)

    # per (node, half) receiver degree
    deg2 = np.bincount(src * 2 + half_tgt, minlength=N * 2).reshape(N, 2)

    # greedy in-order packing per core -> (group, slot) per node
    grp = np.empty(N, np.int64)
    slot = np.empty(N, np.int64)
    for c in range(NC):
        g = 0
        e0 = e1 = cnt = 0
        base = c * NPC
        d = deg2[base:base + NPC]
        for i in range(NPC):
            a0, a1 = d[i]
            if e0 + a0 > CAP or e1 + a1 > CAP or cnt + 1 > NODE_CAP:
                g += 1
                e0 = e1 = cnt = 0
            grp[base + i] = g
            slot[base + i] = cnt
            e0 += a0
            e1 += a1
            cnt += 1
        if g >= G_MAX:
            raise RuntimeError(f"core {c}: {g + 1} groups > G_MAX={G_MAX}")

    core_of = np.arange(N) // NPC
    row_of = core_of * RPC + grp * W_G + slot        # global sigma row per node
    node_of_row = np.full(ROWS, -1, np.int64)
    node_of_row[row_of] = np.arange(N)

    # --- per-edge slot assignment, per (core, half) ---
    # sort edges by (core(src), half(tgt), src) then by edge id for stability
    key = (core_of[src] * 2 + half_tgt) * N + src
    eorder = np.argsort(key, kind="stable")
    s_s, t_s, w_s, h_s = src[eorder], tgt[eorder], ew[eorder], half_tgt[eorder]
    c_s = core_of[s_s]

    # within-node rank (edges of same (src, half) are consecutive)
    nk = key[eorder]
    first = np.r_[True, nk[1:] != nk[:-1]]
    idx_all = np.arange(E)
    start_of_run = np.maximum.accumulate(np.where(first, idx_all, -1))
    rank = idx_all - start_of_run

    # within-group exclusive cumsum of node degrees -> node slot start
    # deg of (node, half) in packing order per group:
    dflat = deg2.reshape(-1)  # index n*2+h
    node_start = np.zeros((N, 2), np.int64)
    for c in range(NC):
        base = c * NPC
        d = deg2[base:base + NPC]
        gg = grp[base:base + NPC]
        for h in (0, 1):
            cum = np.cumsum(d[:, h]) - d[:, h]
            gfirst = np.r_[True, gg[1:] != gg[:-1]]
            gbase = np.maximum.accumulate(np.where(gfirst, cum, -1))
            node_start[base:base + NPC, h] = cum - gbase

    e_slot = grp[s_s] * CAP + node_start[s_s, h_s] + rank     # slot within pass
    e_lane = e_slot % 128
    e_chunk = e_slot // 128                                   # chunk within pass
    e_scol = slot[s_s]                                        # receiver slot in group

    # gather index (int16) relative to table half
    e_idx16 = (row_of[t_s] - h_s * HALF).astype(np.int16)
    assert (row_of[t_s] - h_s * HALF).max() < HALF

    # build per-core arrays
    idx_d = np.full((NC, 2, 128, S_PASS // 16), DUMMY_IDX, np.int16)
    mask_d = np.zeros((NC, 2, 128, (S_PASS // 128) * W_G), np.float32)
    for c in range(NC):
        for h in (0, 1):
            m = (c_s == c) & (h_s == h)
            sl, la, ch, sc = e_slot[m], e_lane[m], e_chunk[m], e_scol[m]
            grid = np.full((16, S_PASS // 16), DUMMY_IDX, np.int16)
            grid[sl % 16, sl // 16] = e_idx16[m]
            idx_d[c, h] = np.tile(grid, (8, 1))
            mask_d[c, h][la, ch * W_G + sc] = w_s[m]

    # degree CSRs in sigma-row order (in: by src/receiver, out: by tgt/sender)
    csr_in = np.zeros((NC, RPC, CSR_W), np.float32)
    csr_out = np.zeros((NC, RPC, CSR_W), np.float32)
    for kind, nd_idx, csr in ((0, src, csr_in), (1, tgt, csr_out)):
        o = np.argsort(nd_idx, kind="stable")
        nds = nd_idx[o]
        firstn = np.r_[True, nds[1:] != nds[:-1]]
        st = np.maximum.accumulate(np.where(firstn, np.arange(E), -1))
        rk = np.arange(E) - st
        assert rk.max() < CSR_W, f"degree {rk.max() + 1} exceeds CSR_W"
        r = row_of[nds]
        csr[r // RPC, r % RPC, rk] = ew[o]

    return idx_d, mask_d, csr_in, csr_out, row_of, node_of_row


# ======================= device program =======================

def _build_program():
    import concourse.bass as bass
    import concourse.bacc as bacc
    import concourse.mybir as mybir
    import concourse.tile as tile
    from concourse.masks import make_identity

    F32 = mybir.dt.float32
    I16 = mybir.dt.int16
    AX = mybir.AxisListType
    OP = mybir.AluOpType
    ACT = mybir.ActivationFunctionType

    nc = bacc.Bacc("TRN2", target_bir_lowering=False, debug=False, num_devices=NC)

    x_sh = nc.dram_tensor("x_sh", [RPC, DIN], F32, kind="ExternalInput")
    csr_i = nc.dram_tensor("csr_i", [RPC, CSR_W], F32, kind="ExternalInput")
    csr_o = nc.dram_tensor("csr_o", [RPC, CSR_W], F32, kind="ExternalInput")
    idx_t = [nc.dram_tensor(f"idx{h}", [128, S_PASS // 16], I16, kind="ExternalInput")
             for h in (0, 1)]
    mask_t = [nc.dram_tensor(f"mask{h}", [128, (S_PASS // 128) * W_G], F32,
                             kind="ExternalInput") for h in (0, 1)]
    w_t = [nc.dram_tensor("w1p", [128, DHID], F32, kind="ExternalInput"),
           nc.dram_tensor("w2p", [128, DHID], F32, kind="ExternalInput"),
           nc.dram_tensor("w3p", [128, DOUT], F32, kind="ExternalInput")]
    h2_out = nc.dram_tensor("h2_out", [RPC, DHID], F32, kind="ExternalOutput")
    pred_out = nc.dram_tensor("pred_out", [RPC, DOUT], F32, kind="ExternalOutput")

    shard = nc.dram_tensor("shard", [RPC, 128], F32, kind="Internal")
    table = nc.dram_tensor("table", [ROWS, 128], F32, kind="Internal",
                           addr_space="Shared")
    RG = [list(range(NC))]

    with tile.TileContext(nc) as tc:
        with (tc.tile_pool(name="const", bufs=1) as cpool,
              tc.tile_pool(name="sb", bufs=3) as pool,
              tc.tile_pool(name="msgp", bufs=2) as msgp,
              tc.tile_pool(name="aggp", bufs=1) as aggp,
              tc.tile_pool(name="ps", bufs=4, space="PSUM") as psp,
              tc.tile_pool(name="ps2", bufs=2, space="PSUM") as psp2):

            ident = cpool.tile([128, 128], F32)
            make_identity(nc, ident[:])
            wt = []
            for i in range(3):
                w = cpool.tile([128, w_t[i].shape[1]], F32, tag=f"w{i}")
                nc.sync.dma_start(w[:], w_t[i][:, :])
                wt.append(w)

            # ---- degrees -> a (rsqrt in-deg), b (rsqrt out-deg), [128, TILES]
            ab = []
            for name, csr in (("a", csr_i), ("b", csr_o)):
                deg = cpool.tile([128, TILES], F32, tag=f"deg_{name}")
                for t in range(TILES):
                    ct = pool.tile([128, CSR_W], F32, tag="csr")
                    nc.sync.dma_start(ct[:], csr[t * 128:(t + 1) * 128, :])
                    nc.vector.reduce_sum(deg[:, t:t + 1], ct[:], axis=AX.X)
                msk = cpool.tile([128, TILES], F32, tag=f"msk_{name}")
                nc.vector.tensor_scalar(out=msk[:], in0=deg[:], scalar1=0.0,
                                        scalar2=None, op0=OP.is_gt)
                nc.vector.tensor_scalar(out=deg[:], in0=deg[:], scalar1=1e-30,
                                        scalar2=None, op0=OP.add)
                nc.vector.reciprocal(deg[:], deg[:])
                nc.scalar.activation(deg[:], deg[:], ACT.Sqrt)
                nc.vector.tensor_tensor(out=deg[:], in0=deg[:], in1=msk[:],
                                        op=OP.mult)
                ab.append(deg)
            a_sb, b_sb = ab

            # ---- initial table shard from x: rows = [b*x | b | 0]
            for t in range(TILES):
                xt = pool.tile([128, DIN], F32, tag="xt")
                nc.sync.dma_start(xt[:], x_sh[t * 128:(t + 1) * 128, :])
                sh = pool.tile([128, 128], F32, tag="sh")
                nc.vector.tensor_scalar(out=sh[:, 0:DIN], in0=xt[:],
                                        scalar1=b_sb[:, t:t + 1], scalar2=None,
                                        op0=OP.mult)
                nc.vector.tensor_copy(sh[:, DIN:DIN + 1], b_sb[:, t:t + 1])
                nc.vector.memset(sh[:, DIN + 1:128], 0.0)
                nc.sync.dma_start(shard[t * 128:(t + 1) * 128, :], sh[:])
            nc.gpsimd.collective_compute(
                "AllGather", mybir.AluOpType.bypass, replica_groups=RG,
                ins=[shard[:]], outs=[table[:]])

            # ---- 3 GCN layers
            for l in range(3):
                odim = DOUT if l == 2 else DHID
                agg = aggp.tile([128, TILES, 97], F32, tag="agg")
                for h in (0, 1):
                    tab_half = table[h * HALF:(h + 1) * HALF, :]
                    for call in range(NCALL):
                        it = pool.tile([128, CALL // 16], I16, tag="idx")
                        nc.sync.dma_start(
                            it[:], idx_t[h][:, call * 512:(call + 1) * 512])
                        mt = pool.tile([128, NCHUNK * W_G], F32, tag="mask")
                        nc.sync.dma_start(
                            mt[:], mask_t[h][:, call * NCHUNK * W_G:
                                             (call + 1) * NCHUNK * W_G])
                        msg = msgp.tile([128, NCHUNK, 128], F32, tag="msg")
                        nc.gpsimd.dma_gather(
                            out_ap=msg[:], in_ap=tab_half, idxs_ap=it[:],
                            num_idxs=CALL, num_idxs_reg=CALL, elem_size=128,
                            single_packet=False)
                        for lp in range(16):       # pairs of groups per call
                            pt = psp.tile([64, 97], F32, tag="ps")
                            for j in (0, 1):
                                lg = lp * 2 + j
                                for k in (0, 1):
                                    ch = lg * 2 + k
                                    nc.tensor.matmul(
                                        out=pt[j * 32:(j + 1) * 32, :],
                                        lhsT=mt[:, ch * W_G:(ch + 1) * W_G],
                                        rhs=msg[:, ch, 0:97],
                                        start=(k == 0), stop=(k == 1))
                            gp = call * 16 + lp    # global pair id
                            dst = agg[(gp % 2) * 64:(gp % 2 + 1) * 64,
                                      gp // 2, :]
                            if h == 0:
                                nc.vector.tensor_copy(dst, pt[:])
                            else:
                                nc.vector.tensor_tensor(
                                    out=dst, in0=dst, in1=pt[:], op=OP.add)

                # epilogue per 128-row tile
                for t in range(TILES):
                    psT = psp2.tile([128, 128], F32, tag="psT")
                    nc.tensor.transpose(psT[0:97, :], agg[:, t, 0:97], ident[:])
                    aT = pool.tile([128, 128], F32, tag="aT")
                    nc.vector.tensor_copy(aT[0:97, :], psT[0:97, :])
                    po = psp2.tile([128, odim], F32, tag="po")
                    nc.tensor.matmul(out=po[:], lhsT=aT[0:97, :],
                                     rhs=wt[l][0:97, 0:odim],
                                     start=True, stop=True)
                    ht = pool.tile([128, odim], F32, tag="ht")
                    nc.scalar.activation(ht[:], po[:], ACT.Relu,
                                         scale=a_sb[:, t:t + 1])
                    if l == 1:
                        nc.sync.dma_start(h2_out[t * 128:(t + 1) * 128, :], ht[:])
                    if l == 2:
                        nc.sync.dma_start(pred_out[t * 128:(t + 1) * 128, :], ht[:])
                    if l < 2:
                        sh = pool.tile([128, 128], F32, tag="sh")
                        nc.vector.tensor_scalar(out=sh[:, 0:DHID], in0=ht[:],
                                                scalar1=b_sb[:, t:t + 1],
                                                scalar2=None, op0=OP.mult)
                        nc.vector.tensor_copy(sh[:, DHID:DHID + 1],
                                              b_sb[:, t:t + 1])
                        nc.vector.memset(sh[:, DHID + 1:128], 0.0)
                        nc.sync.dma_start(shard[t * 128:(t + 1) * 128, :], sh[:])
                if l < 2:
                    nc.gpsimd.collective_compute(
                        "AllGather", mybir.AluOpType.bypass, replica_groups=RG,
                        ins=[shard[:]], outs=[table[:]])

    nc.compile()
    return nc


# ======================= entry point =======================

def _install_ntff_hook():
    """Provide antenv.axon_hooks (missing on this image) so trace=True works."""
    import os, types, ctypes, contextlib
    if "antenv.axon_hooks" in sys.modules:
        return
    hook = None
    so_path = "/opt/axon/libaxon_pjrt.so"
    if os.path.exists(so_path):
        lib = ctypes.CDLL(so_path)
        if hasattr(lib, "axon_start_nrt_profile"):
            lib.axon_start_nrt_profile.argtypes = [
                ctypes.POINTER(ctypes.c_int64), ctypes.c_size_t]
            lib.axon_start_nrt_profile.restype = ctypes.c_int64
            lib.axon_stop_nrt_profile.argtypes = [ctypes.c_char_p]
            lib.axon_stop_nrt_profile.restype = ctypes.c_int64

            @contextlib.contextmanager
            def _hook(output_dir, device_ids):
                import jax
                jax.devices()
                if device_ids:
                    ids = (ctypes.c_int64 * len(device_ids))(*device_ids)
                    rc = lib.axon_start_nrt_profile(ids, len(device_ids))
                else:
                    rc = lib.axon_start_nrt_profile(None, 0)
                if rc != 0:
                    raise RuntimeError(f"axon_start_nrt_profile rc={rc}")
                try:
                    yield
                finally:
                    n = lib.axon_stop_nrt_profile(str(output_dir).encode())
                    if n < 0:
                        raise RuntimeError(f"axon_stop_nrt_profile rc={n}")
            hook = _hook
    import antenv
    mod = types.ModuleType("antenv.axon_hooks")
    mod.get_axon_ntff_profile_hook = lambda: hook
    sys.modules["antenv.axon_hooks"] = mod
    antenv.axon_hooks = mod


def kernel(x, edge_src, edge_tgt, edge_weights, W1, b1, W2, b2, W3, b3,
           _trace=False):
    import concourse.bass_utils as bu
    from concourse.bass_utils import run_bass_kernel_spmd
    if _trace:
        _install_ntff_hook()
        bu.upload_artifacts = lambda d: "(upload skipped)"

    x = np.asarray(x, np.float32)
    idx_d, mask_d, csr_in, csr_out, row_of, node_of_row = _pack(
        edge_src, edge_tgt, edge_weights)

    def packw(W, b):
        W = np.asarray(W, np.float32)
        b = np.asarray(b, np.float32)
        return np.concatenate(
            [W, b[None, :], np.zeros((128 - W.shape[0] - 1, W.shape[1]),
                                     np.float32)], axis=0)

    w1p, w2p, w3p = packw(W1, b1), packw(W2, b2), packw(W3, b3)

    # x rows in sigma order per core
    x_rows = np.zeros((NC, RPC, DIN), np.float32)
    x_rows[row_of // RPC, row_of % RPC] = x

    in_maps = []
    for c in range(NC):
        in_maps.append(dict(
            x_sh=x_rows[c], csr_i=csr_in[c], csr_o=csr_out[c],
            idx0=idx_d[c, 0], idx1=idx_d[c, 1],
            mask0=mask_d[c, 0], mask1=mask_d[c, 1],
            w1p=w1p, w2p=w2p, w3p=w3p))

    if "nc" not in _CACHE:
        _CACHE["nc"] = _build_program()
    nc = _CACHE["nc"]

    res = run_bass_kernel_spmd(nc, in_maps, core_ids=list(range(NC)),
                               trace=_trace)
    if _trace:
        _CACHE["last_results"] = res

    emb = np.zeros((N, DHID), np.float32)
    pred = np.zeros((N, DOUT), np.float32)
    for c in range(NC):
        nr = node_of_row[c * RPC:(c + 1) * RPC]
        m = nr >= 0
        emb[nr[m]] = res.results[c]["h2_out"][m]
        pred[nr[m]] = res.results[c]["pred_out"][m]
    return emb, pred
